# revision 5
# baseline (speedup 1.0000x reference)
"""Differentiable risk budgeting solve on 8 Trainium2 NeuronCores.

Problem: 20 unrolled iterations of
    Sw   = einsum('bij,bj->bi', sigma, w)
    grad = 2*Sw - beta + lam_s*sign(w) + 2*lam_t*(w - w_prev)
    w    = proj(w - 0.05*grad)          # clip/renorm twice
with B=32768, P=45.

Strategy: pure data parallel over 8 cores (4096 batch rows each).
sigma is cast to fp16 on the host and kept entirely SBUF-resident
(~127KiB/partition), so HBM traffic is one half-precision pass.

Per iteration the batched matvec runs as an fp16 elementwise multiply
on the VectorEngine (2x DVE mode for packed 2-byte operands) followed
by an in-place pairwise tree reduction over the contraction axis
(45->23->12->6->3->2->1).  tensor_reduce gets no 16-bit speedup, the
tensor_tensor tree does; its large levels run on the DVE, the small
tail levels on GPSIMD.  The update/projection chain is spread across
the Scalar engine (casts, scaled copies, relu-pair clips with fused
per-group accumulation) and GPSIMD (adds/multiplies, which are the
only ALU ops its ISA accepts), so all three engines overlap across
interleaved batch tiles; the DVE keeps only the sign-term, the
reciprocal, and one tiny fixup per renorm.

Update folded to  u = cw*w - 0.1*Sw - s*sign(w) + D  with
cw = 1-0.1*lam_t, s = 0.05*lam_s, D = 0.05*beta + 0.1*lam_t*w_prev
(host-folded, lambdas baked as immediates), sign(w>=0) realized
branch-free as min(w16*6e4, s) in fp16.  sigma stays UNSCALED in fp16
(products sigma*w ~1e-3 stay in fp16 normal range; pre-scaling by
-0.1 would push them toward subnormals); the -0.1 lands in the fp32
chain via a scaled Act copy.  clip(x,0,.15) = .15 - relu(.15-relu(x))
on Act; the renorm divide becomes q*rr and rr15 - q*rr on GPSIMD with
rr = 1/(6.75 + eps - sum(q)) since sum(.15 - q) = 6.75 - sum(q).
"""

import os
import sys

sys.path.insert(0, "/opt/trn_rl_repo")

import numpy as np

import concourse.bacc as bacc
import concourse.bass as bass
import concourse.mybir as mybir
import concourse.tile as tile
from concourse.bass_utils import run_bass_kernel_spmd

N_CORES = 8
B_TOTAL = 32768
P = 45
BC = B_TOTAL // N_CORES  # 4096 batch rows per core

N_ITER = 20
STEP = 0.05
MAXW = 0.15
EPS = 1e-8
BIGH = 60000.0  # fp16-safe "big": min normal w16 * 6e4 >> s, and 0*6e4 = 0

NB = 4  # batch groups per tile (free dim)
TB = 128 * NB  # batch rows per tile
NT = BC // TB  # tiles per core

TREE_DVE_LEVELS = 3  # first levels (widths 22,11,6) on DVE, tail on GPSIMD

F32 = mybir.dt.float32
F16 = mybir.dt.float16
ALU = mybir.AluOpType
AX = mybir.AxisListType
AF = mybir.ActivationFunctionType


def _tree_steps(n):
    """In-place pairwise halving: a[0:h] += a[n-h:n]; n -> n-h."""
    steps = []
    while n > 1:
        h = n // 2
        steps.append((h, n))
        n -= h
    return steps  # n=45: [(22,45),(11,23),(6,12),(3,6),(1,3),(1,2)]


def _build_program(cw: float, s: float):
    """Trace the per-core Bass program. cw/s are baked as immediates."""
    nc = bacc.Bacc("TRN2", target_bir_lowering=False, debug=False)

    sig_d = nc.dram_tensor("sigma16", [BC, P, P], F16, kind="ExternalInput").ap()
    d_d = nc.dram_tensor("dvec", [BC, P], F32, kind="ExternalInput").ap()
    d0_d = nc.dram_tensor("dvec0", [BC, P], F32, kind="ExternalInput").ap()
    w_d = nc.dram_tensor("wout", [BC, P], F32, kind="ExternalOutput").ap()

    reps = int(os.environ.get("RISK_KERNEL_BENCH_REPS", "1"))

    import contextlib

    steps = _tree_steps(P)
    SUMC = float(np.float32(MAXW) * P + np.float32(EPS))  # 6.75 + eps

    with tile.TileContext(nc) as tc:
        with (
            tc.tile_pool(name="sig", bufs=1) as psig,
            tc.tile_pool(name="prod", bufs=2) as pprod,
            tc.tile_pool(name="wrk", bufs=2) as pwrk,
            tc.For_i(0, reps, 1) if reps > 1 else contextlib.nullcontext(),
        ):
            # bias constant for the second relu of the clip
            b015 = psig.tile([128, 1], F32, tag="b015")
            nc.gpsimd.memset(b015[:], MAXW)

            # ---- resident sigma fp16 + D tiles ----
            sigs, dts, d0s = [], [], []
            for t in range(NT):
                base = t * TB
                sig = psig.tile([128, NB * P * P], F16, tag=f"sig{t}")
                sig4 = sig[:].rearrange("p (g i j) -> p g i j", g=NB, i=P)
                for g in range(NB):
                    nc.gpsimd.dma_start(
                        sig4[:, g], sig_d[base + g * 128 : base + (g + 1) * 128]
                    )
                dt_ = psig.tile([128, NB * P], F32, tag=f"d{t}")
                dt3 = dt_[:].rearrange("p (g j) -> p g j", g=NB)
                d0_ = psig.tile([128, NB * P], F32, tag=f"d0{t}")
                d03 = d0_[:].rearrange("p (g j) -> p g j", g=NB)
                for g in range(NB):
                    nc.gpsimd.dma_start(
                        dt3[:, g], d_d[base + g * 128 : base + (g + 1) * 128]
                    )
                    nc.gpsimd.dma_start(
                        d03[:, g], d0_d[base + g * 128 : base + (g + 1) * 128]
                    )
                sigs.append((sig, sig4))
                dts.append((dt_, dt3))
                d0s.append((d0_, d03))

            for t in range(NT):
                base = t * TB
                sig, sig4 = sigs[t]
                dt_, dt3 = dts[t]
                d0_, d03 = d0s[t]

                w32 = pwrk.tile([128, NB * P], F32, tag="w32")
                w32_3 = w32[:].rearrange("p (g j) -> p g j", g=NB)
                e1 = pwrk.tile([128, NB * P], F32, tag="e1")
                tr_ = pwrk.tile([128, NB * P], F32, tag="tr")
                tcw = pwrk.tile([128, NB * P], F32, tag="tcw")
                tsw = pwrk.tile([128, NB * P], F32, tag="tsw")
                q = pwrk.tile([128, NB * P], F32, tag="q")
                q3 = q[:].rearrange("p (g j) -> p g j", g=NB)
                u2 = pwrk.tile([128, NB * P], F32, tag="u2")
                u2_3 = u2[:].rearrange("p (g j) -> p g j", g=NB)
                r = pwrk.tile([128, NB], F32, tag="r")
                rr = pwrk.tile([128, NB], F32, tag="rr")
                rr15 = pwrk.tile([128, NB], F32, tag="rr15")
                rr_b = rr[:].unsqueeze(2).broadcast_to([128, NB, P])
                rr15_b = rr15[:].unsqueeze(2).broadcast_to([128, NB, P])

                def half_project(src, dst3):
                    # dst = clip(src)/(sum(clip(src))+eps) via q = relu-pair
                    nc.scalar.activation(tr_[:], src, AF.Relu)
                    for g in range(NB):
                        nc.scalar.activation(
                            q3[:, g],
                            tr_[:].rearrange("p (g j) -> p g j", g=NB)[:, g],
                            AF.Relu,
                            b015[:],
                            -1.0,
                            0.0,
                            r[:, g : g + 1],
                        )
                    # rr = 1/(6.75+eps-sum(q)); rr15 = .15*rr
                    nc.vector.tensor_scalar(rr[:], r[:], -1.0, SUMC, ALU.mult, ALU.add)
                    nc.vector.reciprocal(rr[:], rr[:])
                    nc.scalar.activation(rr15[:], rr[:], AF.Copy, 0.0, MAXW)
                    # dst = (0.15 - q) * rr = rr15 - q*rr
                    nc.gpsimd.tensor_tensor(u2_3, q3, rr_b, ALU.mult)
                    nc.gpsimd.tensor_tensor(dst3, rr15_b, u2_3, ALU.subtract)

                def project(src, dst3):
                    half_project(src, e1[:].rearrange("p (g j) -> p g j", g=NB))
                    half_project(e1[:], dst3)

                for it in range(N_ITER):
                    prod = pprod.tile([128, NB * P * P], F16, tag="prod")
                    prod4 = prod[:].rearrange("p (g i j) -> p g i j", g=NB, i=P)

                    if it == 0:
                        # w0 uniform: tree over sigma directly (no multiply)
                        h, n = steps[0]
                        nc.vector.tensor_tensor(
                            prod4[:, :, :, 0:h],
                            sig4[:, :, :, 0:h],
                            sig4[:, :, :, n - h : n],
                            ALU.add,
                        )
                        nc.vector.tensor_copy(
                            prod4[:, :, :, h : n - h], sig4[:, :, :, h : n - h]
                        )
                        rest = steps[1:]
                    else:
                        w16 = pwrk.tile([128, NB * P], F16, tag="w16")
                        nc.scalar.copy(w16[:], w32[:])
                        wb = (
                            w16[:]
                            .rearrange("p (g j) -> p g j", g=NB)
                            .unsqueeze(2)
                            .broadcast_to([128, NB, P, P])
                        )
                        nc.vector.tensor_tensor(prod4, sig4, wb, ALU.mult)
                        rest = steps

                    for li, (h, n) in enumerate(rest):
                        eng = nc.vector if li < TREE_DVE_LEVELS else nc.gpsimd
                        eng.tensor_tensor(
                            prod4[:, :, :, 0:h],
                            prod4[:, :, :, 0:h],
                            prod4[:, :, :, n - h : n],
                            ALU.add,
                        )
                    swp16 = prod4[:, :, :, 0]

                    if it == 0:
                        # u = (-0.1/45)*tree + (cw/45 - s + D)   [dvec0]
                        nc.scalar.activation(
                            tsw[:].rearrange("p (g j) -> p g j", g=NB),
                            swp16,
                            AF.Copy,
                            0.0,
                            -2.0 * STEP / P,
                        )
                        nc.gpsimd.tensor_tensor(e1[:], tsw[:], d0_[:], ALU.add)
                        project(e1[:], w32_3)
                    else:
                        # u = cw*w - s*sign(w) + D - 0.1*tree
                        sgn = pwrk.tile([128, NB * P], F16, tag="sgn")
                        nc.vector.tensor_scalar(
                            sgn[:], w16[:], BIGH, s, ALU.mult, ALU.min
                        )
                        nc.gpsimd.tensor_tensor(tr_[:], dt_[:], sgn[:], ALU.subtract)
                        nc.scalar.activation(tcw[:], w32[:], AF.Copy, 0.0, cw)
                        nc.gpsimd.tensor_tensor(tr_[:], tr_[:], tcw[:], ALU.add)
                        nc.scalar.activation(
                            tsw[:].rearrange("p (g j) -> p g j", g=NB),
                            swp16,
                            AF.Copy,
                            0.0,
                            -2.0 * STEP,
                        )
                        nc.gpsimd.tensor_tensor(e1[:], tr_[:], tsw[:], ALU.add)
                        project(e1[:], w32_3)

                # ---- store ----
                for g in range(NB):
                    nc.gpsimd.dma_start(
                        w_d[base + g * 128 : base + (g + 1) * 128], w32_3[:, g]
                    )

    nc.compile()
    return nc


def _fold(beta, w_prev, log_lambda_sparse, log_lambda_turnover):
    lam_s = np.exp(np.float32(log_lambda_sparse), dtype=np.float32)
    lam_t = np.exp(np.float32(log_lambda_turnover), dtype=np.float32)
    cw = float(np.float32(1.0) - np.float32(2 * STEP) * lam_t)
    s = float(np.float32(STEP) * lam_s)
    dvec = (
        np.float32(STEP) * beta + np.float32(2 * STEP) * lam_t * w_prev
    ).astype(np.float32)
    return cw, s, dvec


def make_in_maps(sigma, beta, w_prev, log_lambda_sparse, log_lambda_turnover):
    cw, s, dvec = _fold(beta, w_prev, log_lambda_sparse, log_lambda_turnover)
    c0 = np.float32(cw) / np.float32(P) - np.float32(s)
    dvec0 = (dvec + c0).astype(np.float32)
    sig16 = np.ascontiguousarray(sigma, dtype=np.float32).astype(np.float16)
    in_maps = []
    for c in range(N_CORES):
        sl = slice(c * BC, (c + 1) * BC)
        in_maps.append(
            {"sigma16": sig16[sl], "dvec": dvec[sl], "dvec0": dvec0[sl]}
        )
    return cw, s, in_maps


def kernel(sigma, beta, w_prev, log_lambda_sparse, log_lambda_turnover):
    beta = np.asarray(beta, dtype=np.float32)
    w_prev = np.asarray(w_prev, dtype=np.float32)
    cw, s, in_maps = make_in_maps(
        sigma, beta, w_prev, log_lambda_sparse, log_lambda_turnover
    )
    nc = _build_program(cw, s)
    res = run_bass_kernel_spmd(nc, in_maps, core_ids=list(range(N_CORES)))
    out = np.concatenate([res.results[c]["wout"] for c in range(N_CORES)], axis=0)
    return out.astype(np.float32)


if __name__ == "__main__":
    rng = np.random.default_rng(0)
    A = rng.standard_normal((B_TOTAL, P, P), dtype=np.float32) * 0.1
    sig = np.einsum("bij,bkj->bik", A, A) + 0.1 * np.eye(P, dtype=np.float32)
    bet = rng.random((B_TOTAL, P), dtype=np.float32)
    bet /= bet.sum(-1, keepdims=True)
    wp = np.full((B_TOTAL, P), 1.0 / P, dtype=np.float32)
    out = kernel(
        sigma=sig,
        beta=bet,
        w_prev=wp,
        log_lambda_sparse=np.float32(-3.0),
        log_lambda_turnover=np.float32(-2.0),
    )
    print(out.shape, out.dtype, out[:2, :5])


# revision 6
# speedup vs baseline: 1.2751x; 1.2751x over previous
"""Differentiable risk budgeting solve on 8 Trainium2 NeuronCores.

Problem: 20 unrolled iterations of
    Sw   = einsum('bij,bj->bi', sigma, w)
    grad = 2*Sw - beta + lam_s*sign(w) + 2*lam_t*(w - w_prev)
    w    = proj(w - 0.05*grad)          # clip/renorm twice
with B=32768, P=45.

Strategy: pure data parallel over 8 cores (4096 batch rows each).
sigma is cast to fp16 on the host and kept entirely SBUF-resident
(~127KiB/partition), so HBM traffic is one half-precision pass.

Per iteration the batched matvec runs on the VectorEngine as an fp16
elementwise multiply (2x DVE perf mode for packed 2-byte operands)
followed by an in-place pairwise tree reduction over the contraction
axis (45->23->12->6->3->2->1) of fp16 tensor_tensor adds -- measured
~3x faster than the mode-less tensor_reduce.  The update/projection
chain splits between the DVE (sign-term, clips, renorm sums,
reciprocals) and GPSIMD (casts, adds, scaled adds, renorm multiplies
-- its ISA only accepts tensor_tensor add/sub/mult and tensor_copy).
The Scalar engine is deliberately unused: measured latency of
dependency-chained Act ops is ~6us each, poison for this serial
chain.  Three batch tiles are kept in flight so the cross-engine
chain of one tile hides under the DVE bulk work of the others.

Update folded to  u = cw*w - 0.1*Sw - s*sign(w) + D  with
cw = 1-0.1*lam_t, s = 0.05*lam_s, D = 0.05*beta + 0.1*lam_t*w_prev
(host-folded, lambdas baked as immediates), sign(w>=0) realized
branch-free as min(w16*6e4, s) in fp16.  sigma stays UNSCALED in fp16
(products sigma*w ~1e-3 stay in fp16 normal range; pre-scaling by
-0.1 would push them toward subnormals); -0.1 and cw enter via
broadcast multiplies against [128,1] constant tiles on GPSIMD.  The
reference's +eps inside renorm shifts results by ~1e-10 relative and
is dropped (the clipped sum is bounded away from zero).
"""

import os
import sys

sys.path.insert(0, "/opt/trn_rl_repo")

import numpy as np

import concourse.bacc as bacc
import concourse.bass as bass
import concourse.mybir as mybir
import concourse.tile as tile
from concourse.bass_utils import run_bass_kernel_spmd

N_CORES = 8
B_TOTAL = 32768
P = 45
BC = B_TOTAL // N_CORES  # 4096 batch rows per core

N_ITER = 20
STEP = 0.05
MAXW = 0.15
EPS = 1e-8
BIGH = 60000.0  # fp16-safe "big": min normal w16 * 6e4 >> s, and 0*6e4 = 0

NB = 4  # batch groups per tile (free dim)
TB = 128 * NB  # batch rows per tile
NT = BC // TB  # tiles per core

F32 = mybir.dt.float32
F16 = mybir.dt.float16
ALU = mybir.AluOpType
AX = mybir.AxisListType


def _tree_steps(n):
    """In-place pairwise halving: a[0:h] += a[n-h:n]; n -> n-h."""
    steps = []
    while n > 1:
        h = n // 2
        steps.append((h, n))
        n -= h
    return steps  # n=45: [(22,45),(11,23),(6,12),(3,6),(1,3),(1,2)]


def _build_program(cw: float, s: float):
    """Trace the per-core Bass program. cw/s are baked as immediates."""
    nc = bacc.Bacc("TRN2", target_bir_lowering=False, debug=False)

    sig_d = nc.dram_tensor("sigma16", [BC, P, P], F16, kind="ExternalInput").ap()
    d_d = nc.dram_tensor("dvec", [BC, P], F32, kind="ExternalInput").ap()
    d0_d = nc.dram_tensor("dvec0", [BC, P], F32, kind="ExternalInput").ap()
    w_d = nc.dram_tensor("wout", [BC, P], F32, kind="ExternalOutput").ap()

    reps = int(os.environ.get("RISK_KERNEL_BENCH_REPS", "1"))

    import contextlib

    steps = _tree_steps(P)

    with tile.TileContext(nc) as tc:
        with (
            tc.tile_pool(name="sig", bufs=1) as psig,
            tc.tile_pool(name="prod", bufs=3) as pprod,
            tc.tile_pool(name="wrk", bufs=3) as pwrk,
            tc.For_i(0, reps, 1) if reps > 1 else contextlib.nullcontext(),
        ):
            # broadcastable [128,1] constants (free-dim broadcast only)
            c_cw = psig.tile([128, 1], F32, tag="c_cw")
            nc.gpsimd.memset(c_cw[:], cw)
            c_m01 = psig.tile([128, 1], F32, tag="c_m01")
            nc.gpsimd.memset(c_m01[:], -2.0 * STEP)
            c_m01p = psig.tile([128, 1], F32, tag="c_m01p")
            nc.gpsimd.memset(c_m01p[:], -2.0 * STEP / P)

            def bc3(t):  # [128,1] -> [128, NB, P] free broadcast
                return t[:].unsqueeze(2).broadcast_to([128, NB, P])

            # ---- resident sigma fp16 + D tiles ----
            sigs, dts, d0s = [], [], []
            for t in range(NT):
                base = t * TB
                sig = psig.tile([128, NB * P * P], F16, tag=f"sig{t}")
                sig4 = sig[:].rearrange("p (g i j) -> p g i j", g=NB, i=P)
                for g in range(NB):
                    nc.gpsimd.dma_start(
                        sig4[:, g], sig_d[base + g * 128 : base + (g + 1) * 128]
                    )
                dt_ = psig.tile([128, NB * P], F32, tag=f"d{t}")
                dt3 = dt_[:].rearrange("p (g j) -> p g j", g=NB)
                d0_ = psig.tile([128, NB * P], F32, tag=f"d0{t}")
                d03 = d0_[:].rearrange("p (g j) -> p g j", g=NB)
                for g in range(NB):
                    nc.gpsimd.dma_start(
                        dt3[:, g], d_d[base + g * 128 : base + (g + 1) * 128]
                    )
                    nc.gpsimd.dma_start(
                        d03[:, g], d0_d[base + g * 128 : base + (g + 1) * 128]
                    )
                sigs.append((sig, sig4))
                dts.append((dt_, dt3))
                d0s.append((d0_, d03))

            for t in range(NT):
                base = t * TB
                sig, sig4 = sigs[t]
                dt_, dt3 = dts[t]
                d0_, d03 = d0s[t]

                w32 = pwrk.tile([128, NB * P], F32, tag="w32")
                w32_3 = w32[:].rearrange("p (g j) -> p g j", g=NB)
                e1 = pwrk.tile([128, NB * P], F32, tag="e1")
                e1_3 = e1[:].rearrange("p (g j) -> p g j", g=NB)
                e2 = pwrk.tile([128, NB * P], F32, tag="e2")
                e2_3 = e2[:].rearrange("p (g j) -> p g j", g=NB)
                wc = pwrk.tile([128, NB * P], F32, tag="wc")
                wc3 = wc[:].rearrange("p (g j) -> p g j", g=NB)
                r = pwrk.tile([128, NB], F32, tag="r")
                rr = pwrk.tile([128, NB], F32, tag="rr")
                rr_b = rr[:].unsqueeze(2).broadcast_to([128, NB, P])

                def half_project(src, dst3):
                    # dst = clip(src)/sum(clip(src))
                    nc.vector.tensor_scalar(wc[:], src, 0.0, MAXW, ALU.max, ALU.min)
                    nc.vector.tensor_reduce(r[:], wc3, AX.X, ALU.add)
                    nc.vector.reciprocal(rr[:], r[:])
                    nc.gpsimd.tensor_tensor(dst3, wc3, rr_b, ALU.mult)

                def project(src, dst3):
                    half_project(src, e2_3)
                    half_project(e2[:], dst3)

                for it in range(N_ITER):
                    prod = pprod.tile([128, NB * P * P], F16, tag="prod")
                    prod4 = prod[:].rearrange("p (g i j) -> p g i j", g=NB, i=P)

                    if it == 0:
                        # w0 uniform: tree over sigma directly (no multiply)
                        h, n = steps[0]
                        nc.vector.tensor_tensor(
                            prod4[:, :, :, 0:h],
                            sig4[:, :, :, 0:h],
                            sig4[:, :, :, n - h : n],
                            ALU.add,
                        )
                        nc.vector.tensor_copy(
                            prod4[:, :, :, h : n - h], sig4[:, :, :, h : n - h]
                        )
                        rest = steps[1:]
                    else:
                        w16 = pwrk.tile([128, NB * P], F16, tag="w16")
                        nc.gpsimd.tensor_copy(w16[:], w32[:])
                        wb = (
                            w16[:]
                            .rearrange("p (g j) -> p g j", g=NB)
                            .unsqueeze(2)
                            .broadcast_to([128, NB, P, P])
                        )
                        nc.vector.tensor_tensor(prod4, sig4, wb, ALU.mult)
                        rest = steps

                    for h, n in rest:
                        nc.vector.tensor_tensor(
                            prod4[:, :, :, 0:h],
                            prod4[:, :, :, 0:h],
                            prod4[:, :, :, n - h : n],
                            ALU.add,
                        )
                    swp16 = prod4[:, :, :, 0]

                    if it == 0:
                        # u = (-0.1/45)*tree + (cw/45 - s + D)   [dvec0]
                        nc.gpsimd.tensor_tensor(
                            e1_3, swp16, bc3(c_m01p), ALU.mult
                        )
                        nc.gpsimd.tensor_tensor(e1[:], e1[:], d0_[:], ALU.add)
                        project(e1[:], w32_3)
                    else:
                        # u = cw*w - s*sign(w) + D - 0.1*tree
                        sgn = pwrk.tile([128, NB * P], F16, tag="sgn")
                        nc.vector.tensor_scalar(
                            sgn[:], w16[:], BIGH, s, ALU.mult, ALU.min
                        )
                        nc.gpsimd.tensor_tensor(e1[:], dt_[:], sgn[:], ALU.subtract)
                        nc.gpsimd.tensor_tensor(wc3, w32_3, bc3(c_cw), ALU.mult)
                        nc.gpsimd.tensor_tensor(e1[:], e1[:], wc[:], ALU.add)
                        nc.gpsimd.tensor_tensor(e2_3, swp16, bc3(c_m01), ALU.mult)
                        nc.gpsimd.tensor_tensor(e1[:], e1[:], e2[:], ALU.add)
                        project(e1[:], w32_3)

                # ---- store ----
                for g in range(NB):
                    nc.gpsimd.dma_start(
                        w_d[base + g * 128 : base + (g + 1) * 128], w32_3[:, g]
                    )

    nc.compile()
    return nc


def _fold(beta, w_prev, log_lambda_sparse, log_lambda_turnover):
    lam_s = np.exp(np.float32(log_lambda_sparse), dtype=np.float32)
    lam_t = np.exp(np.float32(log_lambda_turnover), dtype=np.float32)
    cw = float(np.float32(1.0) - np.float32(2 * STEP) * lam_t)
    s = float(np.float32(STEP) * lam_s)
    dvec = (
        np.float32(STEP) * beta + np.float32(2 * STEP) * lam_t * w_prev
    ).astype(np.float32)
    return cw, s, dvec


def make_in_maps(sigma, beta, w_prev, log_lambda_sparse, log_lambda_turnover):
    cw, s, dvec = _fold(beta, w_prev, log_lambda_sparse, log_lambda_turnover)
    c0 = np.float32(cw) / np.float32(P) - np.float32(s)
    dvec0 = (dvec + c0).astype(np.float32)
    sig16 = np.ascontiguousarray(sigma, dtype=np.float32).astype(np.float16)
    in_maps = []
    for c in range(N_CORES):
        sl = slice(c * BC, (c + 1) * BC)
        in_maps.append(
            {"sigma16": sig16[sl], "dvec": dvec[sl], "dvec0": dvec0[sl]}
        )
    return cw, s, in_maps


def kernel(sigma, beta, w_prev, log_lambda_sparse, log_lambda_turnover):
    beta = np.asarray(beta, dtype=np.float32)
    w_prev = np.asarray(w_prev, dtype=np.float32)
    cw, s, in_maps = make_in_maps(
        sigma, beta, w_prev, log_lambda_sparse, log_lambda_turnover
    )
    nc = _build_program(cw, s)
    res = run_bass_kernel_spmd(nc, in_maps, core_ids=list(range(N_CORES)))
    out = np.concatenate([res.results[c]["wout"] for c in range(N_CORES)], axis=0)
    return out.astype(np.float32)


if __name__ == "__main__":
    rng = np.random.default_rng(0)
    A = rng.standard_normal((B_TOTAL, P, P), dtype=np.float32) * 0.1
    sig = np.einsum("bij,bkj->bik", A, A) + 0.1 * np.eye(P, dtype=np.float32)
    bet = rng.random((B_TOTAL, P), dtype=np.float32)
    bet /= bet.sum(-1, keepdims=True)
    wp = np.full((B_TOTAL, P), 1.0 / P, dtype=np.float32)
    out = kernel(
        sigma=sig,
        beta=bet,
        w_prev=wp,
        log_lambda_sparse=np.float32(-3.0),
        log_lambda_turnover=np.float32(-2.0),
    )
    print(out.shape, out.dtype, out[:2, :5])
